# revision 7
# baseline (speedup 1.0000x reference)
"""Differential Multi-Query Attention — TRN2 Bass kernel, 8-core SPMD.

Sharding: tensor-parallel over the 16 query heads (2 heads per core).
MQA K/V (single head) is computed redundantly on every core. out_proj is
row-parallel: each core computes a partial [S, HID] output from its
256-wide slice of head dims; the all-reduce is the host-side gather sum.

Math notes (exact reformulations of the reference):
  * softmax without max-subtraction (scores ~ N(0,1), no overflow risk):
      a1 = exp(s1)/rowsum(exp(s1))
  * a = a1 - lam*a2 has rowsum exactly (1-lam), so the renorm divisor
    Z = (1-lam)+1e-8 is a constant -> folded into v_w on the host.
  * w = relu(p1/Z1 - lam*p2/Z2) = c1 * relu(p1 + beta*p2) with
    c1 = 1/Z1 > 0 and beta = -lam*Z1/Z2; the c1 scale and the relu fuse
    into one Pool tensor_scalar (mult, max) pass.
  * 1/sqrt(head_dim) folded into q weights on the host (rope is a
    rotation, commutes with scaling).

Device layout: everything flows in "transposed" [feature, seq] form so
the tensor engine (which contracts over the partition dim) never needs
an activation transpose, except the post-relu weights `w` and the v
projection, which are transposed 128x128 on the PE itself (matmul
is_transpose against an identity) -- no DMA/DRAM bounces.

Query blocks are processed in big/small interleaved order
(0,15,1,14,...) so the Act-engine exp load per block stays roughly
constant and the PE never has to wait long for softmax results.
"""

import math
from contextlib import ExitStack

import numpy as np

import concourse.bass as bass
import concourse.bacc as bacc
import concourse.tile as tile
from concourse import mybir
from concourse.bass_utils import run_bass_kernel_spmd

S = 2048          # sequence length
HID = 2048        # hidden dim
HEADS = 16
D = 128           # head dim
NCORES = 8
HPC = HEADS // NCORES   # heads per core = 2
LAM = 0.5
NQB = S // 128    # query blocks of 128
NHT = HID // 128  # hidden-dim tiles of 128

F32 = mybir.dt.float32
BF16 = mybir.dt.bfloat16
AF = mybir.ActivationFunctionType
OP = mybir.AluOpType

NEG_MASK = -1.0e9


def _emit(ctx, tc, xT, wq1, wq2, wk, wv, cosT, sinT, prot, ident, mneg, ow,
          y, klens):
    nc = tc.nc

    # ---------------- persistent tiles ----------------
    persist = ctx.enter_context(tc.tile_pool(name="persist", bufs=1))
    cos_s = persist.tile([128, S], BF16, tag="cos")
    sin_s = persist.tile([128, S], BF16, tag="sin")
    ident_s = persist.tile([128, 128], BF16, tag="ident")
    nc.sync.dma_start(out=cos_s, in_=cosT)
    nc.sync.dma_start(out=sin_s, in_=sinT)
    nc.sync.dma_start(out=ident_s, in_=ident)

    # roped projections, [d=128, S] each (transposed form)
    q1r = [persist.tile([128, S], BF16, name=f"q1r{h}", tag=f"q1r{h}")
           for h in range(HPC)]
    q2r = [persist.tile([128, S], BF16, name=f"q2r{h}", tag=f"q2r{h}")
           for h in range(HPC)]
    kr = persist.tile([128, S], BF16, tag="kr")
    v_nat = persist.tile([128, NQB, 128], BF16, tag="v_nat")  # v natural [k, d]

    # ---------------- stage A: projections + rope ----------------
    NCH = 4
    CW = S // NCH  # 512
    with tc.tile_pool(name="wpool", bufs=1) as wp, \
         tc.tile_pool(name="xpool", bufs=2) as xp, \
         tc.tile_pool(name="ropetmp", bufs=3) as rtp, \
         tc.tile_pool(name="vtp", bufs=1) as vtp, \
         tc.tile_pool(name="projpsum", bufs=2, space="PSUM") as pp, \
         tc.tile_pool(name="vtpsum", bufs=2, space="PSUM") as vpp:
        wq1_s = wp.tile([128, NHT, HPC * D], BF16, tag="wq1")
        wq2_s = wp.tile([128, NHT, HPC * D], BF16, tag="wq2")
        wk_s = wp.tile([128, NHT, D], BF16, tag="wk")
        wv_s = wp.tile([128, NHT, D], BF16, tag="wv")
        prot_s = wp.tile([128, D], BF16, tag="prot")
        nc.sync.dma_start(out=prot_s, in_=prot)
        for dst, srcw in ((wq1_s, wq1), (wq2_s, wq2), (wk_s, wk), (wv_s, wv)):
            nc.sync.dma_start(out=dst, in_=srcw.rearrange("(t p) d -> p t d", p=128))

        vt_bf = vtp.tile([128, S], BF16, tag="vt")
        for c in range(NCH):
            sl = slice(c * CW, (c + 1) * CW)
            xt = xp.tile([128, NHT, CW], BF16, tag="xt")
            xin = xT[:, sl].rearrange("(t p) s -> p t s", p=128)
            nc.sync.dma_start(out=xt, in_=xin)
            targets = []
            for h in range(HPC):
                targets.append((wq1_s, h * D, q1r[h], True))
                targets.append((wq2_s, h * D, q2r[h], True))
            targets.append((wk_s, 0, kr, True))
            targets.append((wv_s, 0, None, False))
            for (w_s, d0, dest, do_rope) in targets:
                ps = pp.tile([128, CW], F32, tag="ps")
                for t in range(NHT):
                    nc.tensor.matmul(
                        ps,
                        lhsT=w_s[:, t, d0:d0 + D],
                        rhs=xt[:, t, :],
                        start=(t == 0),
                        stop=(t == NHT - 1),
                    )
                if do_rope:
                    # rope in [d, s] layout: out = q*cos + rot_half(q)*sin.
                    # rot_half is a signed half-swap along the PARTITION dim;
                    # cross-partition reads are illegal on the vector engines,
                    # so apply it as a signed permutation matmul on PE.
                    q_sb = rtp.tile([128, CW], BF16, tag="qsb")
                    nc.scalar.copy(out=q_sb, in_=ps)
                    rot_ps = pp.tile([128, CW], F32, tag="rot")
                    nc.tensor.matmul(rot_ps, lhsT=prot_s, rhs=q_sb,
                                     start=True, stop=True)
                    m = rtp.tile([128, CW], BF16, tag="m")
                    nc.gpsimd.tensor_tensor(out=m, in0=q_sb, in1=cos_s[:, sl],
                                            op=OP.mult)
                    n = rtp.tile([128, CW], BF16, tag="n")
                    nc.vector.tensor_tensor(out=n, in0=rot_ps, in1=sin_s[:, sl],
                                            op=OP.mult)
                    nc.vector.tensor_tensor(out=dest[:, sl], in0=m, in1=n,
                                            op=OP.add)
                else:
                    nc.scalar.copy(out=vt_bf[:, sl], in_=ps)  # cast f32 -> bf16

        # v^T [d, s] -> v natural [k, d] via PE transposes (128x128 tiles),
        # 8 tiles per PSUM bank, then one DVE copy per bank.
        for g in range(NQB // 8):
            vt_ps = vpp.tile([128, 8, 128], BF16, tag="vtp")
            for j in range(8):
                t = g * 8 + j
                nc.tensor.transpose(
                    vt_ps[:, j, :], vt_bf[:, t * 128:(t + 1) * 128], ident_s)
            nc.vector.tensor_copy(out=v_nat[:, g * 8:(g + 1) * 8, :], in_=vt_ps)

    # ---------------- stage B: attention + out_proj ----------------
    # Query-block order interleaves long and short klens so the exp load
    # on the Act engine stays roughly constant per step.
    qb_order = []
    for i in range(NQB // 2):
        qb_order.append(i)
        qb_order.append(NQB - 1 - i)

    with tc.tile_pool(name="bconst", bufs=1) as bc, \
         tc.tile_pool(name="strips", bufs=3) as sp, \
         tc.tile_pool(name="smallp", bufs=3) as smp, \
         tc.tile_pool(name="atp", bufs=3) as atp, \
         tc.tile_pool(name="attns", bufs=4) as ats, \
         tc.tile_pool(name="ysb", bufs=3) as yp, \
         tc.tile_pool(name="spsum", bufs=4, space="PSUM") as spp, \
         tc.tile_pool(name="tpsum", bufs=2, space="PSUM") as tpp, \
         tc.tile_pool(name="apsum", bufs=2, space="PSUM") as app:
        ow_s = bc.tile([128, HPC, HID], BF16, tag="ow")
        nc.sync.dma_start(out=ow_s, in_=ow.rearrange("(h p) e -> p h e", p=128))
        mneg_s = bc.tile([128, S], BF16, tag="mneg")
        nc.sync.dma_start(out=mneg_s, in_=mneg)

        def emit_out_proj(qb, attnT):
            # row-parallel partial: y[qb] = sum_h attnT_h.T @ ow_h
            # 512-wide chunks; PSUM->SBUF copies alternate Act/DVE.
            ysb = yp.tile([128, HID], BF16, tag="ysb")
            for ec in range(4):
                yps = spp.tile([128, 512], F32, tag="s")
                e0 = ec * 512
                for h in range(HPC):
                    nc.tensor.matmul(
                        yps,
                        lhsT=attnT[h],
                        rhs=ow_s[:, h, e0:e0 + 512],
                        start=(h == 0),
                        stop=(h == HPC - 1),
                    )
                if ec % 2 == 0:
                    nc.scalar.copy(out=ysb[:, e0:e0 + 512], in_=yps)
                else:
                    nc.vector.tensor_copy(out=ysb[:, e0:e0 + 512], in_=yps)
            nc.sync.dma_start(out=y[qb * 128:(qb + 1) * 128, :], in_=ysb)

        pending = None  # (qb, attnT) deferred one block for PE overlap
        for qb in qb_order:
            klen = klens[qb]
            nkt = klen // 128
            nch = (klen + 511) // 512
            # exp row-sum accumulators: [chunk, (s1h0, s1h1, s2h0, s2h1)]
            # (score-major so Z1/Z2 pairs are contiguous column ranges)
            rall = smp.tile([128, 4, 4], F32, tag="rall")
            strips = []
            for h in range(HPC):
                p1 = sp.tile([128, S], BF16, tag="p1")
                p2 = sp.tile([128, S], BF16, tag="p2")
                q1T = q1r[h][:, qb * 128:(qb + 1) * 128]
                q2T = q2r[h][:, qb * 128:(qb + 1) * 128]
                for (si, qT, pstrip) in ((0, q1T, p1), (1, q2T, p2)):
                    for c in range(nch):
                        k0 = c * 512
                        kc = min(512, klen - k0)
                        sps = spp.tile([128, 512], F32, tag="s")
                        nc.tensor.matmul(sps[:, :kc], lhsT=qT,
                                         rhs=kr[:, k0:k0 + kc],
                                         start=True, stop=True)
                        if c == nch - 1:
                            # mask for the last (possibly partial) k block
                            dc = kc - 128
                            nc.vector.tensor_tensor(
                                out=sps[:, dc:dc + 128],
                                in0=sps[:, dc:dc + 128],
                                in1=mneg_s[:, qb * 128:(qb + 1) * 128],
                                op=OP.add,
                            )
                        idx = 2 * si + h
                        nc.scalar.activation(
                            out=pstrip[:, k0:k0 + kc],
                            in_=sps[:, :kc],
                            func=AF.Exp,
                            accum_out=rall[:, c, idx:idx + 1],
                        )
                strips.append((p1, p2))

            # PE bubble-filler: the deferred out_proj of the previous block
            # runs while Act/DVE/Pool chew this block's softmax.
            if pending is not None:
                emit_out_proj(*pending)
                pending = None

            # fold chunk accumulators, then scales:
            #   recips = 1/Z (all four), beta_h = -lam * Z1_h / Z2_h
            zs = rall[:, 0, :]
            for c in range(1, nch):
                nc.vector.tensor_tensor(out=zs, in0=zs, in1=rall[:, c, :],
                                        op=OP.add)
            recips = smp.tile([128, 4], F32, tag="recips")
            nc.vector.reciprocal(out=recips, in_=zs)
            beta = smp.tile([128, 2], F32, tag="beta")
            # zs cols 0:2 = Z1 (both heads); recips cols 2:4 = 1/Z2
            nc.vector.tensor_tensor(
                out=beta, in0=zs[:, 0:2], in1=recips[:, 2:4], op=OP.mult)
            nc.vector.tensor_scalar_mul(beta, beta, -LAM)

            attnT = []
            aT_strips = []
            for h in range(HPC):
                p1, p2 = strips[h]
                # w = c1 * relu(p1 + beta*p2): DVE fused mult-add, then a
                # Pool tensor_scalar (mult, max) applies the positive c1
                # scale inside the relu.
                nc.vector.scalar_tensor_tensor(
                    out=p2[:, :klen], in0=p2[:, :klen],
                    scalar=beta[:, h:h + 1], in1=p1[:, :klen],
                    op0=OP.mult, op1=OP.add,
                )
                w_bf = sp.tile([128, S], BF16, tag="wbf")
                nc.gpsimd.tensor_scalar(
                    out=w_bf[:, :klen], in0=p2[:, :klen],
                    scalar1=recips[:, h:h + 1], scalar2=0.0,
                    op0=OP.mult, op1=OP.max,
                )
                # transpose w via PE (128x128 tiles), 8 per PSUM bank, one
                # DVE copy per bank into the aT strip.
                aT = atp.tile([128, NQB, 128], BF16, tag="aT")
                for g in range((nkt + 7) // 8):
                    gn = min(8, nkt - g * 8)
                    tp_ps = tpp.tile([128, 8, 128], BF16, tag="tp")
                    for j in range(gn):
                        kt = g * 8 + j
                        nc.tensor.transpose(
                            tp_ps[:, j, :],
                            w_bf[:, kt * 128:(kt + 1) * 128], ident_s)
                    nc.vector.tensor_copy(
                        out=aT[:, g * 8:g * 8 + gn, :], in_=tp_ps[:, :gn, :])
                aT_strips.append(aT)
            # deferred a@v: both heads' transposes issue before either matmul
            # group runs, keeping the PE stream dense.
            for h in range(HPC):
                aps = app.tile([128, 128], F32, tag="attn")
                for kt in range(nkt):
                    nc.tensor.matmul(
                        aps,
                        lhsT=v_nat[:, kt, :],
                        rhs=aT_strips[h][:, kt, :],
                        start=(kt == 0),
                        stop=(kt == nkt - 1),
                    )
                at_s = ats.tile([128, 128], BF16, name=f"attnT{h}",
                                tag=f"attnT{h}")
                nc.scalar.copy(out=at_s, in_=aps)
                attnT.append(at_s)
            pending = (qb, attnT)
        emit_out_proj(*pending)


def _build(klens):
    nc = bacc.Bacc("TRN2", target_bir_lowering=False, debug=False)
    xT = nc.dram_tensor("xT", [HID, S], BF16, kind="ExternalInput").ap()
    wq1 = nc.dram_tensor("wq1", [HID, HPC * D], BF16, kind="ExternalInput").ap()
    wq2 = nc.dram_tensor("wq2", [HID, HPC * D], BF16, kind="ExternalInput").ap()
    wk = nc.dram_tensor("wk", [HID, D], BF16, kind="ExternalInput").ap()
    wv = nc.dram_tensor("wv", [HID, D], BF16, kind="ExternalInput").ap()
    cosT = nc.dram_tensor("cosT", [D, S], BF16, kind="ExternalInput").ap()
    sinT = nc.dram_tensor("sinT", [D, S], BF16, kind="ExternalInput").ap()
    prot = nc.dram_tensor("prot", [D, D], BF16, kind="ExternalInput").ap()
    ident = nc.dram_tensor("ident", [D, D], BF16, kind="ExternalInput").ap()
    mneg = nc.dram_tensor("mneg", [128, S], BF16, kind="ExternalInput").ap()
    ow = nc.dram_tensor("ow", [HPC * D, HID], BF16, kind="ExternalInput").ap()
    y = nc.dram_tensor("y", [S, HID], BF16, kind="ExternalOutput").ap()
    with ExitStack() as ctx:
        tc = ctx.enter_context(tile.TileContext(nc))
        _emit(ctx, tc, xT, wq1, wq2, wk, wv, cosT, sinT, prot, ident, mneg,
              ow, y, klens)
    nc.compile()
    return nc


_RUNNER_CACHE = {}
LAST_RUN = None
LAST_EXEC = None  # (runner, dev_args) for timing reuse


class _Runner:
    """Mirrors bass2jax.run_bass_via_pjrt's multi-core path, but caches the
    jitted executable and keeps inputs reusable (no donation) so repeated
    timed executions don't recompile or re-upload."""

    def __init__(self, nc, n_cores):
        import jax
        from jax.sharding import Mesh, PartitionSpec
        from jax.experimental.shard_map import shard_map
        from concourse import bass2jax, mybir as mb

        bass2jax.install_neuronx_cc_hook()
        self.nc = nc
        self.n_cores = n_cores
        partition_name = (
            nc.partition_id_tensor.name if nc.partition_id_tensor else None
        )
        in_names, out_names, out_avals, zero_outs = [], [], [], []
        for alloc in nc.m.functions[0].allocations:
            if not isinstance(alloc, mb.MemoryLocationSet):
                continue
            name = alloc.memorylocations[0].name
            if alloc.kind == "ExternalInput":
                if name != partition_name:
                    in_names.append(name)
            elif alloc.kind == "ExternalOutput":
                out_names.append(name)
                shape = tuple(alloc.tensor_shape)
                dtype = mb.dt.np(alloc.dtype)
                out_avals.append(jax.core.ShapedArray(shape, dtype))
                zero_outs.append(np.zeros(shape, dtype))
        self.in_names = list(in_names)
        self.out_names = out_names
        self.out_avals = out_avals
        self.zero_outs = zero_outs
        n_params = len(in_names)
        all_names = list(in_names + out_names)
        if partition_name is not None:
            all_names.append(partition_name)
        all_names = tuple(all_names)

        def _body(*args):
            operands = list(args)
            if partition_name is not None:
                operands.append(bass2jax.partition_id_tensor())
            outs = bass2jax._bass_exec_p.bind(
                *operands,
                out_avals=tuple(out_avals),
                in_names=all_names,
                out_names=tuple(out_names),
                lowering_input_output_aliases=(),
                sim_require_finite=True,
                sim_require_nnan=True,
                nc=nc,
            )
            return tuple(outs)

        self._body = _body
        devices = jax.devices()[:n_cores]
        self.mesh = Mesh(np.asarray(devices), ("core",))
        self.pspec = PartitionSpec("core")
        in_specs = (self.pspec,) * (n_params + len(out_names))
        out_specs = (self.pspec,) * len(out_names)
        self.fn = jax.jit(
            shard_map(_body, mesh=self.mesh, in_specs=in_specs,
                      out_specs=out_specs, check_rep=False),
            keep_unused=True,
        )

    def loop_fn(self, n):
        """Jitted function executing the kernel n times back-to-back on
        device (effect-ordered). Used to amortize the ~78 ms axon dispatch
        overhead when measuring true HW exec time."""
        import jax
        from jax.experimental.shard_map import shard_map

        if not hasattr(self, "_loop_fns"):
            self._loop_fns = {}
        if n not in self._loop_fns:
            body = self._body

            def _loop(*args):
                outs = None
                for _ in range(n):
                    outs = body(*args)
                return outs

            n_params = len(self.in_names)
            in_specs = (self.pspec,) * (n_params + len(self.out_names))
            out_specs = (self.pspec,) * len(self.out_names)
            self._loop_fns[n] = jax.jit(
                shard_map(_loop, mesh=self.mesh, in_specs=in_specs,
                          out_specs=out_specs, check_rep=False),
                keep_unused=True,
            )
        return self._loop_fns[n]

    def device_args(self, in_maps):
        import jax
        from jax.sharding import NamedSharding

        sharding = NamedSharding(self.mesh, self.pspec)
        concat = [
            np.concatenate([np.asarray(m[name]) for m in in_maps], axis=0)
            for name in self.in_names
        ]
        concat += [
            np.zeros((self.n_cores * z.shape[0], *z.shape[1:]), z.dtype)
            for z in self.zero_outs
        ]
        return [jax.device_put(a, sharding) for a in concat]

    def run(self, dev_args):
        import jax

        outs = self.fn(*dev_args)
        jax.block_until_ready(outs)
        return [
            {
                name: np.asarray(outs[i]).reshape(
                    self.n_cores, *self.out_avals[i].shape)[c]
                for i, name in enumerate(self.out_names)
            }
            for c in range(self.n_cores)
        ]


def _get_runner(klens):
    key = tuple(klens)
    if key not in _RUNNER_CACHE:
        _RUNNER_CACHE[key] = _Runner(_build(klens), NCORES)
    return _RUNNER_CACHE[key]


def measure_hw(n_long=96, n_short=8, reps=4):
    """True per-execution HW time via loop amortization: run the kernel
    n_long and n_short times in single dispatches; the slope removes the
    ~78 ms axon dispatch overhead."""
    import time
    import jax

    runner, dev_args = LAST_EXEC
    f_long = runner.loop_fn(n_long)
    f_short = runner.loop_fn(n_short)

    def timed(f):
        best = float("inf")
        for _ in range(reps):
            t0 = time.perf_counter()
            outs = f(*dev_args)
            jax.block_until_ready(outs)
            best = min(best, time.perf_counter() - t0)
        return best

    timed(f_short)  # warm both paths
    timed(f_long)
    t_short = timed(f_short)
    t_long = timed(f_long)
    per_exec = (t_long - t_short) / (n_long - n_short)
    if per_exec <= 0:
        per_exec = t_long / n_long  # upper bound when RPC noise dominates
    return per_exec, t_short, t_long


def _prep_mask(mask):
    """Per query-block: attended k extent (klen) and the additive mask for
    the last 128-wide k block. Requires every non-final block in range to
    be all-True (holds for causal and for all-ones masks)."""
    mask = np.asarray(mask).astype(bool)
    klens = []
    mneg = np.zeros((128, S), np.float32)
    for qb in range(NQB):
        rows = mask[qb * 128:(qb + 1) * 128, :]
        any_col = rows.any(axis=0)
        assert any_col.any(), f"query block {qb} attends nothing"
        last = int(np.nonzero(any_col)[0][-1])
        nkt = last // 128 + 1
        klen = nkt * 128
        klens.append(klen)
        blk = rows[:, (nkt - 1) * 128:klen]
        mneg[:, qb * 128:(qb + 1) * 128] = np.where(blk, 0.0, NEG_MASK)
        inner = rows[:, :(nkt - 1) * 128]
        if not inner.all():
            raise NotImplementedError(
                "mask has partial blocks before the final attended block; "
                "only causal / all-ones style masks are supported"
            )
    return klens, mneg


def host_prep(x, freqs_cos, freqs_sin, mask, q1_w, q2_w, k_w, v_w, out_w):
    """Host-side input marshalling: transpose/fold/shard. Returns
    (klens, in_maps)."""
    x = np.asarray(x, np.float32)
    assert x.shape == (1, S, HID)
    xT = np.ascontiguousarray(x[0].T)
    scale = 1.0 / math.sqrt(D)
    Z = (1.0 - LAM) + 1e-8

    cosT = np.ascontiguousarray(np.asarray(freqs_cos, np.float32).T)
    sinT = np.ascontiguousarray(np.asarray(freqs_sin, np.float32).T)
    # signed rotate-half as a matmul: rot = protM @ q with
    # protM[d, d+64] = -1 (d<64), protM[d, d-64] = +1 (d>=64); lhsT = protM.T
    protM = np.zeros((D, D), np.float32)
    for d in range(64):
        protM[d, d + 64] = -1.0
        protM[d + 64, d] = 1.0
    protT = np.ascontiguousarray(protM.T)

    klens, mneg = _prep_mask(mask)

    q1_w = np.asarray(q1_w, np.float32) * scale
    q2_w = np.asarray(q2_w, np.float32) * scale
    k_w = np.ascontiguousarray(np.asarray(k_w, np.float32))
    v_w = np.ascontiguousarray(np.asarray(v_w, np.float32) / Z)
    out_w = np.asarray(out_w, np.float32)

    import ml_dtypes
    bf = ml_dtypes.bfloat16
    xT = xT.astype(bf)
    k_w = k_w.astype(bf)
    v_w = v_w.astype(bf)
    protT = protT.astype(bf)
    identM = np.eye(D, dtype=np.float32).astype(bf)
    in_maps = []
    for c in range(NCORES):
        h0 = c * HPC * D
        in_maps.append({
            "xT": xT,
            "wq1": np.ascontiguousarray(q1_w[:, h0:h0 + HPC * D]).astype(bf),
            "wq2": np.ascontiguousarray(q2_w[:, h0:h0 + HPC * D]).astype(bf),
            "wk": k_w,
            "wv": v_w,
            "cosT": cosT.astype(bf),
            "sinT": sinT.astype(bf),
            "prot": protT,
            "ident": identM,
            "mneg": mneg.astype(bf),
            "ow": np.ascontiguousarray(out_w[h0:h0 + HPC * D, :]).astype(bf),
        })
    return klens, in_maps


def kernel(x, freqs_cos, freqs_sin, mask, q1_w, q2_w, k_w, v_w, out_w):
    global LAST_RUN, LAST_EXEC
    klens, in_maps = host_prep(
        x, freqs_cos, freqs_sin, mask, q1_w, q2_w, k_w, v_w, out_w)
    runner = _get_runner(klens)
    dev_args = runner.device_args(in_maps)
    LAST_EXEC = (runner, dev_args)
    results = runner.run(dev_args)
    LAST_RUN = results
    y = results[0]["y"].astype(np.float32)
    for c in range(1, NCORES):
        y = y + results[c]["y"].astype(np.float32)
    return y.reshape(1, S, HID)


# revision 10
# speedup vs baseline: 2.4307x; 2.4307x over previous
"""Differential Multi-Query Attention — TRN2 Bass kernel, 8-core SPMD.

Sharding: tensor-parallel over the 16 query heads (2 heads per core).
MQA K/V (single head) is computed redundantly on every core. out_proj is
row-parallel: each core computes a partial [S, HID] output from its
256-wide slice of head dims; the all-reduce is the host-side gather sum.

Math notes (exact reformulations of the reference):
  * softmax without max-subtraction (scores ~ N(0,1), no overflow risk):
      a1 = exp(s1)/rowsum(exp(s1))
  * a = a1 - lam*a2 has rowsum exactly (1-lam), so the renorm divisor
    Z = (1-lam)+1e-8 is a constant -> folded into v_w on the host.
  * w = relu(p1/Z1 - lam*p2/Z2) = c1 * relu(p1 + beta*p2) with
    c1 = 1/Z1 > 0 and beta = -lam*Z1/Z2; the c1 scale and the relu fuse
    into one Pool tensor_scalar (mult, max) pass.
  * 1/sqrt(head_dim) folded into q weights on the host (rope is a
    rotation, commutes with scaling).

Device layout: everything flows in "transposed" [feature, seq] form so
the tensor engine (which contracts over the partition dim) never needs
an activation transpose, except the post-relu weights `w` and the v
projection, which are transposed 128x128 on the PE itself (matmul
is_transpose against an identity) -- no DMA/DRAM bounces.

Query blocks are processed in big/small interleaved order
(0,15,1,14,...) so the Act-engine exp load per block stays roughly
constant and the PE never has to wait long for softmax results.
"""

import math
from contextlib import ExitStack

import numpy as np

import concourse.bass as bass
import concourse.bacc as bacc
import concourse.tile as tile
from concourse import mybir
from concourse.bass_utils import run_bass_kernel_spmd

S = 2048          # sequence length
HID = 2048        # hidden dim
HEADS = 16
D = 128           # head dim
NCORES = 8
HPC = HEADS // NCORES   # heads per core = 2
LAM = 0.5
NQB = S // 128    # query blocks of 128
NHT = HID // 128  # hidden-dim tiles of 128

F32 = mybir.dt.float32
BF16 = mybir.dt.bfloat16
AF = mybir.ActivationFunctionType
OP = mybir.AluOpType

NEG_MASK = -1.0e9


def _emit(ctx, tc, xT, wq1, wq2, wk, wv, cosT, sinT, prot, ident, mneg, ow,
          y, klens):
    nc = tc.nc

    # ---------------- persistent tiles ----------------
    persist = ctx.enter_context(tc.tile_pool(name="persist", bufs=1))
    cos_s = persist.tile([128, S], BF16, tag="cos")
    sin_s = persist.tile([128, S], BF16, tag="sin")
    ident_s = persist.tile([128, 128], BF16, tag="ident")
    nc.sync.dma_start(out=cos_s, in_=cosT)
    nc.sync.dma_start(out=sin_s, in_=sinT)
    nc.sync.dma_start(out=ident_s, in_=ident)

    # roped projections, [d=128, S] each (transposed form)
    q1r = [persist.tile([128, S], BF16, name=f"q1r{h}", tag=f"q1r{h}")
           for h in range(HPC)]
    q2r = [persist.tile([128, S], BF16, name=f"q2r{h}", tag=f"q2r{h}")
           for h in range(HPC)]
    kr = persist.tile([128, S], BF16, tag="kr")
    v_nat = persist.tile([128, NQB, 128], BF16, tag="v_nat")  # v natural [k, d]

    # ---------------- stage A: projections + rope ----------------
    NCH = 4
    CW = S // NCH  # 512
    with tc.tile_pool(name="wpool", bufs=1) as wp, \
         tc.tile_pool(name="xpool", bufs=2) as xp, \
         tc.tile_pool(name="ropetmp", bufs=3) as rtp, \
         tc.tile_pool(name="vtp", bufs=1) as vtp, \
         tc.tile_pool(name="projpsum", bufs=2, space="PSUM") as pp, \
         tc.tile_pool(name="vtpsum", bufs=2, space="PSUM") as vpp:
        wq1_s = wp.tile([128, NHT, HPC * D], BF16, tag="wq1")
        wq2_s = wp.tile([128, NHT, HPC * D], BF16, tag="wq2")
        wk_s = wp.tile([128, NHT, D], BF16, tag="wk")
        wv_s = wp.tile([128, NHT, D], BF16, tag="wv")
        prot_s = wp.tile([128, D], BF16, tag="prot")
        nc.sync.dma_start(out=prot_s, in_=prot)
        for dst, srcw in ((wq1_s, wq1), (wq2_s, wq2), (wk_s, wk), (wv_s, wv)):
            wsrc = srcw.rearrange("(t p) d -> p t d", p=128)
            for g in range(0, NHT, 8):
                nc.sync.dma_start(out=dst[:, g:g + 8, :], in_=wsrc[:, g:g + 8, :])

        vt_bf = vtp.tile([128, S], BF16, tag="vt")
        for c in range(NCH):
            sl = slice(c * CW, (c + 1) * CW)
            xt = xp.tile([128, NHT, CW], BF16, tag="xt")
            xin = xT[:, sl].rearrange("(t p) s -> p t s", p=128)
            for g in range(0, NHT, 4):
                nc.sync.dma_start(out=xt[:, g:g + 4, :], in_=xin[:, g:g + 4, :])
            targets = []
            for h in range(HPC):
                targets.append((wq1_s, h * D, q1r[h], True))
                targets.append((wq2_s, h * D, q2r[h], True))
            targets.append((wk_s, 0, kr, True))
            targets.append((wv_s, 0, None, False))
            for (w_s, d0, dest, do_rope) in targets:
                ps = pp.tile([128, CW], F32, tag="ps")
                for t in range(NHT):
                    nc.tensor.matmul(
                        ps,
                        lhsT=w_s[:, t, d0:d0 + D],
                        rhs=xt[:, t, :],
                        start=(t == 0),
                        stop=(t == NHT - 1),
                    )
                if do_rope:
                    # rope in [d, s] layout: out = q*cos + rot_half(q)*sin.
                    # rot_half is a signed half-swap along the PARTITION dim;
                    # cross-partition reads are illegal on the vector engines,
                    # so apply it as a signed permutation matmul on PE.
                    q_sb = rtp.tile([128, CW], BF16, tag="qsb")
                    nc.scalar.copy(out=q_sb, in_=ps)
                    rot_ps = pp.tile([128, CW], F32, tag="rot")
                    nc.tensor.matmul(rot_ps, lhsT=prot_s, rhs=q_sb,
                                     start=True, stop=True)
                    m = rtp.tile([128, CW], BF16, tag="m")
                    nc.vector.tensor_tensor(out=m, in0=q_sb, in1=cos_s[:, sl],
                                            op=OP.mult)
                    n = rtp.tile([128, CW], BF16, tag="n")
                    nc.vector.tensor_tensor(out=n, in0=rot_ps, in1=sin_s[:, sl],
                                            op=OP.mult)
                    nc.vector.tensor_tensor(out=dest[:, sl], in0=m, in1=n,
                                            op=OP.add)
                else:
                    nc.scalar.copy(out=vt_bf[:, sl], in_=ps)  # cast f32 -> bf16

        # v^T [d, s] -> v natural [k, d] via PE transposes (128x128 tiles),
        # 8 tiles per PSUM bank, then one DVE copy per bank.
        for g in range(NQB // 8):
            vt_ps = vpp.tile([128, 8, 128], BF16, tag="vtp")
            for j in range(8):
                t = g * 8 + j
                nc.tensor.transpose(
                    vt_ps[:, j, :], vt_bf[:, t * 128:(t + 1) * 128], ident_s)
            nc.vector.tensor_copy(out=v_nat[:, g * 8:(g + 1) * 8, :], in_=vt_ps)

    # ---------------- stage B: attention + out_proj ----------------
    # Query-block order interleaves long and short klens so the exp load
    # on the Act engine stays roughly constant per step.
    qb_order = []
    for i in range(NQB // 2):
        qb_order.append(i)
        qb_order.append(NQB - 1 - i)

    with tc.tile_pool(name="bconst", bufs=1) as bc, \
         tc.tile_pool(name="strips", bufs=3) as sp, \
         tc.tile_pool(name="smallp", bufs=3) as smp, \
         tc.tile_pool(name="atp", bufs=3) as atp, \
         tc.tile_pool(name="attns", bufs=4) as ats, \
         tc.tile_pool(name="ysb", bufs=3) as yp, \
         tc.tile_pool(name="spsum", bufs=4, space="PSUM") as spp, \
         tc.tile_pool(name="tpsum", bufs=2, space="PSUM") as tpp, \
         tc.tile_pool(name="apsum", bufs=2, space="PSUM") as app:
        ow_s = bc.tile([128, HPC, HID], BF16, tag="ow")
        nc.sync.dma_start(out=ow_s, in_=ow.rearrange("(h p) e -> p h e", p=128))
        mneg_s = bc.tile([128, S], BF16, tag="mneg")
        nc.sync.dma_start(out=mneg_s, in_=mneg)

        def emit_out_proj(qb, attnT):
            # row-parallel partial: y[qb] = sum_h attnT_h.T @ ow_h
            # 512-wide chunks; PSUM->SBUF copies alternate Act/DVE.
            ysb = yp.tile([128, HID], BF16, tag="ysb")
            for ec in range(4):
                yps = spp.tile([128, 512], F32, tag="s")
                e0 = ec * 512
                for h in range(HPC):
                    nc.tensor.matmul(
                        yps,
                        lhsT=attnT[h],
                        rhs=ow_s[:, h, e0:e0 + 512],
                        start=(h == 0),
                        stop=(h == HPC - 1),
                    )
                if ec % 2 == 0:
                    nc.scalar.copy(out=ysb[:, e0:e0 + 512], in_=yps)
                else:
                    nc.vector.tensor_copy(out=ysb[:, e0:e0 + 512], in_=yps)
            nc.sync.dma_start(out=y[qb * 128:(qb + 1) * 128, :], in_=ysb)

        pending = None  # (qb, attnT) deferred one block for PE overlap
        for qb in qb_order:
            klen = klens[qb]
            nkt = klen // 128
            nch = (klen + 511) // 512
            # exp row-sum accumulators: [chunk, (s1h0, s1h1, s2h0, s2h1)]
            # (score-major so Z1/Z2 pairs are contiguous column ranges)
            rall = smp.tile([128, 4, 4], F32, tag="rall")
            strips = []
            for h in range(HPC):
                p1 = sp.tile([128, S], BF16, tag="p1")
                p2 = sp.tile([128, S], BF16, tag="p2")
                q1T = q1r[h][:, qb * 128:(qb + 1) * 128]
                q2T = q2r[h][:, qb * 128:(qb + 1) * 128]
                for (si, qT, pstrip) in ((0, q1T, p1), (1, q2T, p2)):
                    for c in range(nch):
                        k0 = c * 512
                        kc = min(512, klen - k0)
                        sps = spp.tile([128, 512], F32, tag="s")
                        nc.tensor.matmul(sps[:, :kc], lhsT=qT,
                                         rhs=kr[:, k0:k0 + kc],
                                         start=True, stop=True)
                        if c == nch - 1:
                            # mask for the last (possibly partial) k block
                            dc = kc - 128
                            nc.vector.tensor_tensor(
                                out=sps[:, dc:dc + 128],
                                in0=sps[:, dc:dc + 128],
                                in1=mneg_s[:, qb * 128:(qb + 1) * 128],
                                op=OP.add,
                            )
                        idx = 2 * si + h
                        nc.scalar.activation(
                            out=pstrip[:, k0:k0 + kc],
                            in_=sps[:, :kc],
                            func=AF.Exp,
                            accum_out=rall[:, c, idx:idx + 1],
                        )
                strips.append((p1, p2))

            # PE bubble-filler: the deferred out_proj of the previous block
            # runs while Act/DVE/Pool chew this block's softmax.
            if pending is not None:
                emit_out_proj(*pending)
                pending = None

            # fold chunk accumulators, then scales:
            #   recips = 1/Z (all four), beta_h = -lam * Z1_h / Z2_h
            zs = rall[:, 0, :]
            for c in range(1, nch):
                nc.vector.tensor_tensor(out=zs, in0=zs, in1=rall[:, c, :],
                                        op=OP.add)
            recips = smp.tile([128, 4], F32, tag="recips")
            nc.vector.reciprocal(out=recips, in_=zs)
            beta = smp.tile([128, 2], F32, tag="beta")
            # zs cols 0:2 = Z1 (both heads); recips cols 2:4 = 1/Z2
            nc.vector.tensor_tensor(
                out=beta, in0=zs[:, 0:2], in1=recips[:, 2:4], op=OP.mult)
            nc.vector.tensor_scalar_mul(beta, beta, -LAM)

            attnT = []
            aT_strips = []
            for h in range(HPC):
                p1, p2 = strips[h]
                w_bf = sp.tile([128, S], BF16, tag="wbf")
                aT = atp.tile([128, NQB, 128], BF16, tag="aT")
                # w = c1 * relu(p1 + beta*p2), pipelined in 1024-wide chunks:
                # DVE fused mult-add, DVE tensor_scalar (mult, max) applies
                # the positive c1 scale inside the relu (4x packed mode),
                # then PE transposes the chunk's 8 k-tiles into a PSUM bank
                # and one DVE copy lands them in the aT strip.
                for g in range((nkt + 7) // 8):
                    g0 = g * 1024
                    gw = min(1024, klen - g0)
                    gn = (gw + 127) // 128
                    nc.vector.scalar_tensor_tensor(
                        out=p2[:, g0:g0 + gw], in0=p2[:, g0:g0 + gw],
                        scalar=beta[:, h:h + 1], in1=p1[:, g0:g0 + gw],
                        op0=OP.mult, op1=OP.add,
                    )
                    nc.vector.tensor_scalar(
                        out=w_bf[:, g0:g0 + gw], in0=p2[:, g0:g0 + gw],
                        scalar1=recips[:, h:h + 1], scalar2=0.0,
                        op0=OP.mult, op1=OP.max,
                    )
                    tp_ps = tpp.tile([128, 8, 128], BF16, tag="tp")
                    for j in range(gn):
                        kt = g * 8 + j
                        nc.tensor.transpose(
                            tp_ps[:, j, :],
                            w_bf[:, kt * 128:(kt + 1) * 128], ident_s)
                    nc.vector.tensor_copy(
                        out=aT[:, g * 8:g * 8 + gn, :], in_=tp_ps[:, :gn, :])
                aT_strips.append(aT)
            # deferred a@v: both heads' transposes issue before either matmul
            # group runs, keeping the PE stream dense.
            for h in range(HPC):
                aps = app.tile([128, 128], F32, tag="attn")
                for kt in range(nkt):
                    nc.tensor.matmul(
                        aps,
                        lhsT=v_nat[:, kt, :],
                        rhs=aT_strips[h][:, kt, :],
                        start=(kt == 0),
                        stop=(kt == nkt - 1),
                    )
                at_s = ats.tile([128, 128], BF16, name=f"attnT{h}",
                                tag=f"attnT{h}")
                nc.scalar.copy(out=at_s, in_=aps)
                attnT.append(at_s)
            pending = (qb, attnT)
        emit_out_proj(*pending)


def _build(klens):
    nc = bacc.Bacc("TRN2", target_bir_lowering=False, debug=False)
    xT = nc.dram_tensor("xT", [HID, S], BF16, kind="ExternalInput").ap()
    wq1 = nc.dram_tensor("wq1", [HID, HPC * D], BF16, kind="ExternalInput").ap()
    wq2 = nc.dram_tensor("wq2", [HID, HPC * D], BF16, kind="ExternalInput").ap()
    wk = nc.dram_tensor("wk", [HID, D], BF16, kind="ExternalInput").ap()
    wv = nc.dram_tensor("wv", [HID, D], BF16, kind="ExternalInput").ap()
    cosT = nc.dram_tensor("cosT", [D, S], BF16, kind="ExternalInput").ap()
    sinT = nc.dram_tensor("sinT", [D, S], BF16, kind="ExternalInput").ap()
    prot = nc.dram_tensor("prot", [D, D], BF16, kind="ExternalInput").ap()
    ident = nc.dram_tensor("ident", [D, D], BF16, kind="ExternalInput").ap()
    mneg = nc.dram_tensor("mneg", [128, S], BF16, kind="ExternalInput").ap()
    ow = nc.dram_tensor("ow", [HPC * D, HID], BF16, kind="ExternalInput").ap()
    y = nc.dram_tensor("y", [S, HID], BF16, kind="ExternalOutput").ap()
    with ExitStack() as ctx:
        tc = ctx.enter_context(tile.TileContext(nc))
        _emit(ctx, tc, xT, wq1, wq2, wk, wv, cosT, sinT, prot, ident, mneg,
              ow, y, klens)
    nc.compile()
    return nc


_RUNNER_CACHE = {}
LAST_RUN = None
LAST_EXEC = None  # (runner, dev_args) for timing reuse


class _Runner:
    """Mirrors bass2jax.run_bass_via_pjrt's multi-core path, but caches the
    jitted executable and keeps inputs reusable (no donation) so repeated
    timed executions don't recompile or re-upload."""

    def __init__(self, nc, n_cores):
        import jax
        from jax.sharding import Mesh, PartitionSpec
        from jax.experimental.shard_map import shard_map
        from concourse import bass2jax, mybir as mb

        bass2jax.install_neuronx_cc_hook()
        self.nc = nc
        self.n_cores = n_cores
        partition_name = (
            nc.partition_id_tensor.name if nc.partition_id_tensor else None
        )
        in_names, out_names, out_avals, zero_outs = [], [], [], []
        for alloc in nc.m.functions[0].allocations:
            if not isinstance(alloc, mb.MemoryLocationSet):
                continue
            name = alloc.memorylocations[0].name
            if alloc.kind == "ExternalInput":
                if name != partition_name:
                    in_names.append(name)
            elif alloc.kind == "ExternalOutput":
                out_names.append(name)
                shape = tuple(alloc.tensor_shape)
                dtype = mb.dt.np(alloc.dtype)
                out_avals.append(jax.core.ShapedArray(shape, dtype))
                zero_outs.append(np.zeros(shape, dtype))
        self.in_names = list(in_names)
        self.out_names = out_names
        self.out_avals = out_avals
        self.zero_outs = zero_outs
        n_params = len(in_names)
        all_names = list(in_names + out_names)
        if partition_name is not None:
            all_names.append(partition_name)
        all_names = tuple(all_names)

        def _body(*args):
            operands = list(args)
            if partition_name is not None:
                operands.append(bass2jax.partition_id_tensor())
            outs = bass2jax._bass_exec_p.bind(
                *operands,
                out_avals=tuple(out_avals),
                in_names=all_names,
                out_names=tuple(out_names),
                lowering_input_output_aliases=(),
                sim_require_finite=True,
                sim_require_nnan=True,
                nc=nc,
            )
            return tuple(outs)

        self._body = _body
        devices = jax.devices()[:n_cores]
        self.mesh = Mesh(np.asarray(devices), ("core",))
        self.pspec = PartitionSpec("core")
        in_specs = (self.pspec,) * (n_params + len(out_names))
        out_specs = (self.pspec,) * len(out_names)
        self.fn = jax.jit(
            shard_map(_body, mesh=self.mesh, in_specs=in_specs,
                      out_specs=out_specs, check_rep=False),
            keep_unused=True,
        )

    def loop_fn(self, n):
        """Jitted function executing the kernel n times back-to-back on
        device (effect-ordered). Used to amortize the ~78 ms axon dispatch
        overhead when measuring true HW exec time."""
        import jax
        from jax.experimental.shard_map import shard_map

        if not hasattr(self, "_loop_fns"):
            self._loop_fns = {}
        if n not in self._loop_fns:
            body = self._body

            def _loop(*args):
                outs = None
                for _ in range(n):
                    outs = body(*args)
                return outs

            n_params = len(self.in_names)
            in_specs = (self.pspec,) * (n_params + len(self.out_names))
            out_specs = (self.pspec,) * len(self.out_names)
            self._loop_fns[n] = jax.jit(
                shard_map(_loop, mesh=self.mesh, in_specs=in_specs,
                          out_specs=out_specs, check_rep=False),
                keep_unused=True,
            )
        return self._loop_fns[n]

    def device_args(self, in_maps):
        import jax
        from jax.sharding import NamedSharding

        sharding = NamedSharding(self.mesh, self.pspec)
        concat = [
            np.concatenate([np.asarray(m[name]) for m in in_maps], axis=0)
            for name in self.in_names
        ]
        concat += [
            np.zeros((self.n_cores * z.shape[0], *z.shape[1:]), z.dtype)
            for z in self.zero_outs
        ]
        return [jax.device_put(a, sharding) for a in concat]

    def run(self, dev_args):
        import jax

        outs = self.fn(*dev_args)
        jax.block_until_ready(outs)
        return [
            {
                name: np.asarray(outs[i]).reshape(
                    self.n_cores, *self.out_avals[i].shape)[c]
                for i, name in enumerate(self.out_names)
            }
            for c in range(self.n_cores)
        ]


def _get_runner(klens):
    key = tuple(klens)
    if key not in _RUNNER_CACHE:
        _RUNNER_CACHE[key] = _Runner(_build(klens), NCORES)
    return _RUNNER_CACHE[key]


def measure_hw(n_long=96, n_short=8, reps=4):
    """True per-execution HW time via loop amortization: run the kernel
    n_long and n_short times in single dispatches; the slope removes the
    ~78 ms axon dispatch overhead."""
    import time
    import jax

    runner, dev_args = LAST_EXEC
    f_long = runner.loop_fn(n_long)
    f_short = runner.loop_fn(n_short)

    def timed(f):
        best = float("inf")
        for _ in range(reps):
            t0 = time.perf_counter()
            outs = f(*dev_args)
            jax.block_until_ready(outs)
            best = min(best, time.perf_counter() - t0)
        return best

    timed(f_short)  # warm both paths
    timed(f_long)
    t_short = timed(f_short)
    t_long = timed(f_long)
    per_exec = (t_long - t_short) / (n_long - n_short)
    if per_exec <= 0:
        per_exec = t_long / n_long  # upper bound when RPC noise dominates
    return per_exec, t_short, t_long


def _prep_mask(mask):
    """Per query-block: attended k extent (klen) and the additive mask for
    the last 128-wide k block. Requires every non-final block in range to
    be all-True (holds for causal and for all-ones masks)."""
    mask = np.asarray(mask).astype(bool)
    klens = []
    mneg = np.zeros((128, S), np.float32)
    for qb in range(NQB):
        rows = mask[qb * 128:(qb + 1) * 128, :]
        any_col = rows.any(axis=0)
        assert any_col.any(), f"query block {qb} attends nothing"
        last = int(np.nonzero(any_col)[0][-1])
        nkt = last // 128 + 1
        klen = nkt * 128
        klens.append(klen)
        blk = rows[:, (nkt - 1) * 128:klen]
        mneg[:, qb * 128:(qb + 1) * 128] = np.where(blk, 0.0, NEG_MASK)
        inner = rows[:, :(nkt - 1) * 128]
        if not inner.all():
            raise NotImplementedError(
                "mask has partial blocks before the final attended block; "
                "only causal / all-ones style masks are supported"
            )
    return klens, mneg


def host_prep(x, freqs_cos, freqs_sin, mask, q1_w, q2_w, k_w, v_w, out_w):
    """Host-side input marshalling: transpose/fold/shard. Returns
    (klens, in_maps)."""
    x = np.asarray(x, np.float32)
    assert x.shape == (1, S, HID)
    xT = np.ascontiguousarray(x[0].T)
    scale = 1.0 / math.sqrt(D)
    Z = (1.0 - LAM) + 1e-8

    cosT = np.ascontiguousarray(np.asarray(freqs_cos, np.float32).T)
    sinT = np.ascontiguousarray(np.asarray(freqs_sin, np.float32).T)
    # signed rotate-half as a matmul: rot = protM @ q with
    # protM[d, d+64] = -1 (d<64), protM[d, d-64] = +1 (d>=64); lhsT = protM.T
    protM = np.zeros((D, D), np.float32)
    for d in range(64):
        protM[d, d + 64] = -1.0
        protM[d + 64, d] = 1.0
    protT = np.ascontiguousarray(protM.T)

    klens, mneg = _prep_mask(mask)

    q1_w = np.asarray(q1_w, np.float32) * scale
    q2_w = np.asarray(q2_w, np.float32) * scale
    k_w = np.ascontiguousarray(np.asarray(k_w, np.float32))
    v_w = np.ascontiguousarray(np.asarray(v_w, np.float32) / Z)
    out_w = np.asarray(out_w, np.float32)

    import ml_dtypes
    bf = ml_dtypes.bfloat16
    xT = xT.astype(bf)
    k_w = k_w.astype(bf)
    v_w = v_w.astype(bf)
    protT = protT.astype(bf)
    identM = np.eye(D, dtype=np.float32).astype(bf)
    in_maps = []
    for c in range(NCORES):
        h0 = c * HPC * D
        in_maps.append({
            "xT": xT,
            "wq1": np.ascontiguousarray(q1_w[:, h0:h0 + HPC * D]).astype(bf),
            "wq2": np.ascontiguousarray(q2_w[:, h0:h0 + HPC * D]).astype(bf),
            "wk": k_w,
            "wv": v_w,
            "cosT": cosT.astype(bf),
            "sinT": sinT.astype(bf),
            "prot": protT,
            "ident": identM,
            "mneg": mneg.astype(bf),
            "ow": np.ascontiguousarray(out_w[h0:h0 + HPC * D, :]).astype(bf),
        })
    return klens, in_maps


def kernel(x, freqs_cos, freqs_sin, mask, q1_w, q2_w, k_w, v_w, out_w):
    global LAST_RUN, LAST_EXEC
    klens, in_maps = host_prep(
        x, freqs_cos, freqs_sin, mask, q1_w, q2_w, k_w, v_w, out_w)
    runner = _get_runner(klens)
    dev_args = runner.device_args(in_maps)
    LAST_EXEC = (runner, dev_args)
    results = runner.run(dev_args)
    LAST_RUN = results
    y = results[0]["y"].astype(np.float32)
    for c in range(1, NCORES):
        y = y + results[c]["y"].astype(np.float32)
    return y.reshape(1, S, HID)


# revision 16
# speedup vs baseline: 2.6815x; 1.1032x over previous
"""Differential Multi-Query Attention — TRN2 Bass kernel, 8-core SPMD.

Sharding: tensor-parallel over the 16 query heads (2 heads per core).
MQA K/V (single head) is computed redundantly on every core. out_proj is
row-parallel: each core computes a partial [S, HID] output from its
256-wide slice of head dims; the all-reduce is the host-side gather sum.

Math notes (exact reformulations of the reference):
  * softmax without max-subtraction (scores ~ N(0,1), no overflow risk):
      a1 = exp(s1)/rowsum(exp(s1))
  * a = a1 - lam*a2 has rowsum exactly (1-lam), so the renorm divisor
    Z = (1-lam)+1e-8 is a constant -> folded into v_w on the host.
  * w = relu(p1/Z1 - lam*p2/Z2) = c1 * relu(p1 + beta*p2) with
    c1 = 1/Z1 > 0 and beta = -lam*Z1/Z2; the c1 scale and the relu fuse
    into one Pool tensor_scalar (mult, max) pass.
  * 1/sqrt(head_dim) folded into q weights on the host (rope is a
    rotation, commutes with scaling).

Device layout: everything flows in "transposed" [feature, seq] form so
the tensor engine (which contracts over the partition dim) never needs
an activation transpose, except the post-relu weights `w` and the v
projection, which are transposed 128x128 on the PE itself (matmul
is_transpose against an identity) -- no DMA/DRAM bounces.

Query blocks are processed in big/small interleaved order
(0,15,1,14,...) so the Act-engine exp load per block stays roughly
constant and the PE never has to wait long for softmax results.
"""

import math
from contextlib import ExitStack

import numpy as np

import concourse.bass as bass
import concourse.bacc as bacc
import concourse.tile as tile
from concourse import mybir
from concourse.bass_utils import run_bass_kernel_spmd



S = 2048          # sequence length
HID = 2048        # hidden dim
HEADS = 16
D = 128           # head dim
NCORES = 8
HPC = HEADS // NCORES   # heads per core = 2
LAM = 0.5
NQB = S // 128    # query blocks of 128
NHT = HID // 128  # hidden-dim tiles of 128

F32 = mybir.dt.float32
BF16 = mybir.dt.bfloat16
AF = mybir.ActivationFunctionType
OP = mybir.AluOpType

NEG_MASK = -1.0e9


def _emit(ctx, tc, xT, wq1, wq2, wk, wv, cosT, sinT, prot, ident, mneg, ow,
          y, klens):
    nc = tc.nc

    # ---------------- persistent tiles ----------------
    persist = ctx.enter_context(tc.tile_pool(name="persist", bufs=1))
    cos_s = persist.tile([128, S], BF16, tag="cos")
    sin_s = persist.tile([128, S], BF16, tag="sin")
    ident_s = persist.tile([128, 128], BF16, tag="ident")
    nc.sync.dma_start(out=cos_s, in_=cosT)
    nc.sync.dma_start(out=sin_s, in_=sinT)
    nc.sync.dma_start(out=ident_s, in_=ident)

    # roped projections, [d=128, S] each (transposed form)
    q1r = [persist.tile([128, S], BF16, name=f"q1r{h}", tag=f"q1r{h}")
           for h in range(HPC)]
    q2r = [persist.tile([128, S], BF16, name=f"q2r{h}", tag=f"q2r{h}")
           for h in range(HPC)]
    kr = persist.tile([128, S], BF16, tag="kr")
    v_nat = persist.tile([128, NQB, 128], BF16, tag="v_nat")  # v natural [k, d]

    # ---------------- stage A: projections + rope ----------------
    NCH = 4
    CW = S // NCH  # 512
    with tc.tile_pool(name="wpool", bufs=1) as wp, \
         tc.tile_pool(name="xpool", bufs=2) as xp, \
         tc.tile_pool(name="ropetmp", bufs=3) as rtp, \
         tc.tile_pool(name="vtp", bufs=1) as vtp, \
         tc.tile_pool(name="projpsum", bufs=2, space="PSUM") as pp, \
         tc.tile_pool(name="vtpsum", bufs=2, space="PSUM") as vpp:
        wq1_s = wp.tile([128, NHT, HPC * D], BF16, tag="wq1")
        wq2_s = wp.tile([128, NHT, HPC * D], BF16, tag="wq2")
        wk_s = wp.tile([128, NHT, D], BF16, tag="wk")
        wv_s = wp.tile([128, NHT, D], BF16, tag="wv")
        prot_s = wp.tile([128, D], BF16, tag="prot")
        nc.sync.dma_start(out=prot_s, in_=prot)
        for dst, srcw in ((wq1_s, wq1), (wq2_s, wq2), (wk_s, wk), (wv_s, wv)):
            wsrc = srcw.rearrange("(t p) d -> p t d", p=128)
            for g in range(0, NHT, 8):
                nc.sync.dma_start(out=dst[:, g:g + 8, :], in_=wsrc[:, g:g + 8, :])

        vt_bf = vtp.tile([128, S], BF16, tag="vt")
        for c in range(NCH):
            sl = slice(c * CW, (c + 1) * CW)
            xt = xp.tile([128, NHT, CW], BF16, tag="xt")
            xin = xT[:, sl].rearrange("(t p) s -> p t s", p=128)
            for g in range(0, NHT, 4):
                nc.sync.dma_start(out=xt[:, g:g + 4, :], in_=xin[:, g:g + 4, :])
            targets = []
            for h in range(HPC):
                targets.append((wq1_s, h * D, q1r[h], True))
                targets.append((wq2_s, h * D, q2r[h], True))
            targets.append((wk_s, 0, kr, True))
            targets.append((wv_s, 0, None, False))
            for (w_s, d0, dest, do_rope) in targets:
                ps = pp.tile([128, CW], F32, tag="ps")
                for t in range(NHT):
                    nc.tensor.matmul(
                        ps,
                        lhsT=w_s[:, t, d0:d0 + D],
                        rhs=xt[:, t, :],
                        start=(t == 0),
                        stop=(t == NHT - 1),
                    )
                if do_rope:
                    # rope in [d, s] layout: out = q*cos + rot_half(q)*sin.
                    # rot_half is a signed half-swap along the PARTITION dim;
                    # cross-partition reads are illegal on the vector engines,
                    # so apply it as a signed permutation matmul on PE.
                    q_sb = rtp.tile([128, CW], BF16, tag="qsb")
                    nc.scalar.copy(out=q_sb, in_=ps)
                    rot_ps = pp.tile([128, CW], F32, tag="rot")
                    nc.tensor.matmul(rot_ps, lhsT=prot_s, rhs=q_sb,
                                     start=True, stop=True)
                    m = rtp.tile([128, CW], BF16, tag="m")
                    nc.vector.tensor_tensor(out=m, in0=q_sb, in1=cos_s[:, sl],
                                            op=OP.mult)
                    n = rtp.tile([128, CW], BF16, tag="n")
                    nc.vector.tensor_tensor(out=n, in0=rot_ps, in1=sin_s[:, sl],
                                            op=OP.mult)
                    nc.vector.tensor_tensor(out=dest[:, sl], in0=m, in1=n,
                                            op=OP.add)
                else:
                    nc.scalar.copy(out=vt_bf[:, sl], in_=ps)  # cast f32 -> bf16

        # v^T [d, s] -> v natural [k, d] via PE transposes (128x128 tiles),
        # 8 tiles per PSUM bank, then one DVE copy per bank.
        for g in range(NQB // 8):
            vt_ps = vpp.tile([128, 8, 128], BF16, tag="vtp")
            for j in range(8):
                t = g * 8 + j
                nc.tensor.transpose(
                    vt_ps[:, j, :], vt_bf[:, t * 128:(t + 1) * 128], ident_s)
            nc.vector.tensor_copy(out=v_nat[:, g * 8:(g + 1) * 8, :], in_=vt_ps)

    # ---------------- stage B: attention + out_proj ----------------
    # Query-block order interleaves long and short klens so the exp load
    # on the Act engine stays roughly constant per step.
    qb_order = []
    for i in range(NQB // 2):
        qb_order.append(i)
        qb_order.append(NQB - 1 - i)

    with tc.tile_pool(name="bconst", bufs=1) as bc, \
         tc.tile_pool(name="strips", bufs=3) as sp, \
         tc.tile_pool(name="smallp", bufs=3) as smp, \
         tc.tile_pool(name="atp", bufs=3) as atp, \
         tc.tile_pool(name="attns", bufs=4) as ats, \
         tc.tile_pool(name="ysb", bufs=3) as yp, \
         tc.tile_pool(name="spsum", bufs=5, space="PSUM") as spp, \
         tc.tile_pool(name="tpsum", bufs=2, space="PSUM") as tpp, \
         tc.tile_pool(name="apsum", bufs=1, space="PSUM") as app:
        ow_s = bc.tile([128, HPC, HID], BF16, tag="ow")
        nc.sync.dma_start(out=ow_s, in_=ow.rearrange("(h p) e -> p h e", p=128))
        mneg_s = bc.tile([128, S], BF16, tag="mneg")
        nc.sync.dma_start(out=mneg_s, in_=mneg)

        def emit_out_proj(qb, attnT):
            # row-parallel partial: y[qb] = sum_h attnT_h.T @ ow_h
            # 512-wide chunks; PSUM->SBUF copies alternate Act/DVE.
            ysb = yp.tile([128, HID], BF16, tag="ysb")
            for ec in range(4):
                yps = spp.tile([128, 512], F32, tag="s")
                e0 = ec * 512
                for h in range(HPC):
                    nc.tensor.matmul(
                        yps,
                        lhsT=attnT[h],
                        rhs=ow_s[:, h, e0:e0 + 512],
                        start=(h == 0),
                        stop=(h == HPC - 1),
                    )
                if ec % 2 == 0:
                    nc.scalar.copy(out=ysb[:, e0:e0 + 512], in_=yps)
                else:
                    nc.vector.tensor_copy(out=ysb[:, e0:e0 + 512], in_=yps)
            nc.sync.dma_start(out=y[qb * 128:(qb + 1) * 128, :], in_=ysb)

        pending = None  # (qb, attnT) deferred one block for PE overlap
        for qb in qb_order:
            klen = klens[qb]
            nkt = klen // 128
            nch = (klen + 511) // 512
            strips = []
            for h in range(HPC):
                p1 = sp.tile([128, S], BF16, tag="p1")
                p2 = sp.tile([128, S], BF16, tag="p2")
                # per-head exp row-sum accumulators: [chunk, (s1, s2)] --
                # per-head so head h's scales are ready as soon as its own
                # exps land (head h+1's scores still running on PE).
                rall = smp.tile([128, 4, 2], F32, name=f"rall{h}",
                                tag=f"rall{h}")
                q1T = q1r[h][:, qb * 128:(qb + 1) * 128]
                q2T = q2r[h][:, qb * 128:(qb + 1) * 128]
                for (si, qT, pstrip) in ((0, q1T, p1), (1, q2T, p2)):
                    for c in range(nch):
                        k0 = c * 512
                        kc = min(512, klen - k0)
                        sps = spp.tile([128, 512], F32, tag="s")
                        nc.tensor.matmul(sps[:, :kc], lhsT=qT,
                                         rhs=kr[:, k0:k0 + kc],
                                         start=True, stop=True)
                        if c == nch - 1:
                            # mask for the last (possibly partial) k block
                            dc = kc - 128
                            nc.vector.tensor_tensor(
                                out=sps[:, dc:dc + 128],
                                in0=sps[:, dc:dc + 128],
                                in1=mneg_s[:, qb * 128:(qb + 1) * 128],
                                op=OP.add,
                            )
                        nc.scalar.activation(
                            out=pstrip[:, k0:k0 + kc],
                            in_=sps[:, :kc],
                            func=AF.Exp,
                            accum_out=rall[:, c, si:si + 1],
                        )
                # fold chunk accumulators, then scales:
                #   recip_h = (1/Z1, 1/Z2), beta_h = -lam * Z1 / Z2
                zs = rall[:, 0, :]
                for c in range(1, nch):
                    nc.vector.tensor_tensor(out=zs, in0=zs,
                                            in1=rall[:, c, :], op=OP.add)
                recip_h = smp.tile([128, 2], F32, name=f"recip{h}",
                                   tag=f"recip{h}")
                nc.vector.reciprocal(out=recip_h, in_=zs)
                beta_h = smp.tile([128, 1], F32, name=f"beta{h}",
                                  tag=f"beta{h}")
                nc.vector.tensor_tensor(
                    out=beta_h, in0=zs[:, 0:1], in1=recip_h[:, 1:2],
                    op=OP.mult)
                nc.vector.tensor_scalar_mul(beta_h, beta_h, -LAM)
                strips.append((p1, p2, recip_h, beta_h))

            # PE bubble-filler: the deferred out_proj of the previous block
            # runs while Act/DVE chew this block's softmax.
            if pending is not None:
                emit_out_proj(*pending)
                pending = None

            attnT = []
            aT_strips = []
            for h in range(HPC):
                p1, p2, recip_h, beta_h = strips[h]
                w_bf = sp.tile([128, S], BF16, tag="wbf")
                aT = atp.tile([128, NQB, 128], BF16, tag="aT")
                # w = c1 * relu(p1 + beta*p2), pipelined in 1024-wide chunks:
                # DVE fused mult-add, DVE tensor_scalar (mult, max) applies
                # the positive c1 scale inside the relu (4x packed mode),
                # then PE transposes the chunk's 8 k-tiles into a PSUM bank
                # and one DVE copy lands them in the aT strip.
                for g in range((nkt + 7) // 8):
                    g0 = g * 1024
                    gw = min(1024, klen - g0)
                    gn = (gw + 127) // 128
                    nc.vector.scalar_tensor_tensor(
                        out=p2[:, g0:g0 + gw], in0=p2[:, g0:g0 + gw],
                        scalar=beta_h, in1=p1[:, g0:g0 + gw],
                        op0=OP.mult, op1=OP.add,
                    )
                    nc.vector.tensor_scalar(
                        out=w_bf[:, g0:g0 + gw], in0=p2[:, g0:g0 + gw],
                        scalar1=recip_h[:, 0:1], scalar2=0.0,
                        op0=OP.mult, op1=OP.max,
                    )
                    tp_ps = tpp.tile([128, 8, 128], BF16, tag="tp")
                    for j in range(gn):
                        kt = g * 8 + j
                        nc.tensor.transpose(
                            tp_ps[:, j, :],
                            w_bf[:, kt * 128:(kt + 1) * 128], ident_s)
                    nc.vector.tensor_copy(
                        out=aT[:, g * 8:g * 8 + gn, :], in_=tp_ps[:, :gn, :])
                aT_strips.append(aT)
            # deferred a@v: both heads' transposes issue before either matmul
            # group runs, keeping the PE stream dense.
            for h in range(HPC):
                aps = app.tile([128, 128], F32, tag="attn")
                for kt in range(nkt):
                    nc.tensor.matmul(
                        aps,
                        lhsT=v_nat[:, kt, :],
                        rhs=aT_strips[h][:, kt, :],
                        start=(kt == 0),
                        stop=(kt == nkt - 1),
                    )
                at_s = ats.tile([128, 128], BF16, name=f"attnT{h}",
                                tag=f"attnT{h}")
                nc.vector.tensor_copy(out=at_s, in_=aps)
                attnT.append(at_s)
            pending = (qb, attnT)
        emit_out_proj(*pending)


def _build(klens):
    nc = bacc.Bacc("TRN2", target_bir_lowering=False, debug=False)
    xT = nc.dram_tensor("xT", [HID, S], BF16, kind="ExternalInput").ap()
    wq1 = nc.dram_tensor("wq1", [HID, HPC * D], BF16, kind="ExternalInput").ap()
    wq2 = nc.dram_tensor("wq2", [HID, HPC * D], BF16, kind="ExternalInput").ap()
    wk = nc.dram_tensor("wk", [HID, D], BF16, kind="ExternalInput").ap()
    wv = nc.dram_tensor("wv", [HID, D], BF16, kind="ExternalInput").ap()
    cosT = nc.dram_tensor("cosT", [D, S], BF16, kind="ExternalInput").ap()
    sinT = nc.dram_tensor("sinT", [D, S], BF16, kind="ExternalInput").ap()
    prot = nc.dram_tensor("prot", [D, D], BF16, kind="ExternalInput").ap()
    ident = nc.dram_tensor("ident", [D, D], BF16, kind="ExternalInput").ap()
    mneg = nc.dram_tensor("mneg", [128, S], BF16, kind="ExternalInput").ap()
    ow = nc.dram_tensor("ow", [HPC * D, HID], BF16, kind="ExternalInput").ap()
    y = nc.dram_tensor("y", [S, HID], BF16, kind="ExternalOutput").ap()
    with ExitStack() as ctx:
        tc = ctx.enter_context(tile.TileContext(nc))
        _emit(ctx, tc, xT, wq1, wq2, wk, wv, cosT, sinT, prot, ident, mneg,
              ow, y, klens)
    nc.compile()
    return nc


_RUNNER_CACHE = {}
LAST_RUN = None
LAST_EXEC = None  # (runner, dev_args) for timing reuse


class _Runner:
    """Mirrors bass2jax.run_bass_via_pjrt's multi-core path, but caches the
    jitted executable and keeps inputs reusable (no donation) so repeated
    timed executions don't recompile or re-upload."""

    def __init__(self, nc, n_cores):
        import jax
        from jax.sharding import Mesh, PartitionSpec
        from jax.experimental.shard_map import shard_map
        from concourse import bass2jax, mybir as mb

        bass2jax.install_neuronx_cc_hook()
        self.nc = nc
        self.n_cores = n_cores
        partition_name = (
            nc.partition_id_tensor.name if nc.partition_id_tensor else None
        )
        in_names, out_names, out_avals, zero_outs = [], [], [], []
        for alloc in nc.m.functions[0].allocations:
            if not isinstance(alloc, mb.MemoryLocationSet):
                continue
            name = alloc.memorylocations[0].name
            if alloc.kind == "ExternalInput":
                if name != partition_name:
                    in_names.append(name)
            elif alloc.kind == "ExternalOutput":
                out_names.append(name)
                shape = tuple(alloc.tensor_shape)
                dtype = mb.dt.np(alloc.dtype)
                out_avals.append(jax.core.ShapedArray(shape, dtype))
                zero_outs.append(np.zeros(shape, dtype))
        self.in_names = list(in_names)
        self.out_names = out_names
        self.out_avals = out_avals
        self.zero_outs = zero_outs
        n_params = len(in_names)
        all_names = list(in_names + out_names)
        if partition_name is not None:
            all_names.append(partition_name)
        all_names = tuple(all_names)

        def _body(*args):
            operands = list(args)
            if partition_name is not None:
                operands.append(bass2jax.partition_id_tensor())
            outs = bass2jax._bass_exec_p.bind(
                *operands,
                out_avals=tuple(out_avals),
                in_names=all_names,
                out_names=tuple(out_names),
                lowering_input_output_aliases=(),
                sim_require_finite=True,
                sim_require_nnan=True,
                nc=nc,
            )
            return tuple(outs)

        self._body = _body
        devices = jax.devices()[:n_cores]
        self.mesh = Mesh(np.asarray(devices), ("core",))
        self.pspec = PartitionSpec("core")
        in_specs = (self.pspec,) * (n_params + len(out_names))
        out_specs = (self.pspec,) * len(out_names)
        self.fn = jax.jit(
            shard_map(_body, mesh=self.mesh, in_specs=in_specs,
                      out_specs=out_specs, check_rep=False),
            keep_unused=True,
        )

    def loop_fn(self, n):
        """Jitted function executing the kernel n times back-to-back on
        device (effect-ordered). Used to amortize the ~78 ms axon dispatch
        overhead when measuring true HW exec time."""
        import jax
        from jax.experimental.shard_map import shard_map

        if not hasattr(self, "_loop_fns"):
            self._loop_fns = {}
        if n not in self._loop_fns:
            body = self._body

            def _loop(*args):
                outs = None
                for _ in range(n):
                    outs = body(*args)
                return outs

            n_params = len(self.in_names)
            in_specs = (self.pspec,) * (n_params + len(self.out_names))
            out_specs = (self.pspec,) * len(self.out_names)
            self._loop_fns[n] = jax.jit(
                shard_map(_loop, mesh=self.mesh, in_specs=in_specs,
                          out_specs=out_specs, check_rep=False),
                keep_unused=True,
            )
        return self._loop_fns[n]

    def device_args(self, in_maps):
        import jax
        from jax.sharding import NamedSharding

        sharding = NamedSharding(self.mesh, self.pspec)
        concat = [
            np.concatenate([np.asarray(m[name]) for m in in_maps], axis=0)
            for name in self.in_names
        ]
        concat += [
            np.zeros((self.n_cores * z.shape[0], *z.shape[1:]), z.dtype)
            for z in self.zero_outs
        ]
        return [jax.device_put(a, sharding) for a in concat]

    def run(self, dev_args):
        import jax

        outs = self.fn(*dev_args)
        jax.block_until_ready(outs)
        return [
            {
                name: np.asarray(outs[i]).reshape(
                    self.n_cores, *self.out_avals[i].shape)[c]
                for i, name in enumerate(self.out_names)
            }
            for c in range(self.n_cores)
        ]


def _get_runner(klens):
    key = tuple(klens)
    if key not in _RUNNER_CACHE:
        _RUNNER_CACHE[key] = _Runner(_build(klens), NCORES)
    return _RUNNER_CACHE[key]


def measure_hw(n_long=96, n_short=8, reps=4):
    """True per-execution HW time via loop amortization: run the kernel
    n_long and n_short times in single dispatches; the slope removes the
    ~78 ms axon dispatch overhead."""
    import time
    import jax

    runner, dev_args = LAST_EXEC
    f_long = runner.loop_fn(n_long)
    f_short = runner.loop_fn(n_short)

    def timed(f):
        best = float("inf")
        for _ in range(reps):
            t0 = time.perf_counter()
            outs = f(*dev_args)
            jax.block_until_ready(outs)
            best = min(best, time.perf_counter() - t0)
        return best

    timed(f_short)  # warm both paths
    timed(f_long)
    t_short = timed(f_short)
    t_long = timed(f_long)
    per_exec = (t_long - t_short) / (n_long - n_short)
    if per_exec <= 0:
        per_exec = t_long / n_long  # upper bound when RPC noise dominates
    return per_exec, t_short, t_long


def _prep_mask(mask):
    """Per query-block: attended k extent (klen) and the additive mask for
    the last 128-wide k block. Requires every non-final block in range to
    be all-True (holds for causal and for all-ones masks)."""
    mask = np.asarray(mask).astype(bool)
    klens = []
    mneg = np.zeros((128, S), np.float32)
    for qb in range(NQB):
        rows = mask[qb * 128:(qb + 1) * 128, :]
        any_col = rows.any(axis=0)
        assert any_col.any(), f"query block {qb} attends nothing"
        last = int(np.nonzero(any_col)[0][-1])
        nkt = last // 128 + 1
        klen = nkt * 128
        klens.append(klen)
        blk = rows[:, (nkt - 1) * 128:klen]
        mneg[:, qb * 128:(qb + 1) * 128] = np.where(blk, 0.0, NEG_MASK)
        inner = rows[:, :(nkt - 1) * 128]
        if not inner.all():
            raise NotImplementedError(
                "mask has partial blocks before the final attended block; "
                "only causal / all-ones style masks are supported"
            )
    return klens, mneg


def host_prep(x, freqs_cos, freqs_sin, mask, q1_w, q2_w, k_w, v_w, out_w):
    """Host-side input marshalling: transpose/fold/shard. Returns
    (klens, in_maps)."""
    x = np.asarray(x, np.float32)
    assert x.shape == (1, S, HID)
    xT = np.ascontiguousarray(x[0].T)
    scale = 1.0 / math.sqrt(D)
    Z = (1.0 - LAM) + 1e-8

    cosT = np.ascontiguousarray(np.asarray(freqs_cos, np.float32).T)
    sinT = np.ascontiguousarray(np.asarray(freqs_sin, np.float32).T)
    # signed rotate-half as a matmul: rot = protM @ q with
    # protM[d, d+64] = -1 (d<64), protM[d, d-64] = +1 (d>=64); lhsT = protM.T
    protM = np.zeros((D, D), np.float32)
    for d in range(64):
        protM[d, d + 64] = -1.0
        protM[d + 64, d] = 1.0
    protT = np.ascontiguousarray(protM.T)

    klens, mneg = _prep_mask(mask)

    q1_w = np.asarray(q1_w, np.float32) * scale
    q2_w = np.asarray(q2_w, np.float32) * scale
    k_w = np.ascontiguousarray(np.asarray(k_w, np.float32))
    v_w = np.ascontiguousarray(np.asarray(v_w, np.float32) / Z)
    out_w = np.asarray(out_w, np.float32)

    import ml_dtypes
    bf = ml_dtypes.bfloat16
    xT = xT.astype(bf)
    k_w = k_w.astype(bf)
    v_w = v_w.astype(bf)
    protT = protT.astype(bf)
    identM = np.eye(D, dtype=np.float32).astype(bf)
    in_maps = []
    for c in range(NCORES):
        h0 = c * HPC * D
        in_maps.append({
            "xT": xT,
            "wq1": np.ascontiguousarray(q1_w[:, h0:h0 + HPC * D]).astype(bf),
            "wq2": np.ascontiguousarray(q2_w[:, h0:h0 + HPC * D]).astype(bf),
            "wk": k_w,
            "wv": v_w,
            "cosT": cosT.astype(bf),
            "sinT": sinT.astype(bf),
            "prot": protT,
            "ident": identM,
            "mneg": mneg.astype(bf),
            "ow": np.ascontiguousarray(out_w[h0:h0 + HPC * D, :]).astype(bf),
        })
    return klens, in_maps


def kernel(x, freqs_cos, freqs_sin, mask, q1_w, q2_w, k_w, v_w, out_w):
    global LAST_RUN, LAST_EXEC
    klens, in_maps = host_prep(
        x, freqs_cos, freqs_sin, mask, q1_w, q2_w, k_w, v_w, out_w)
    runner = _get_runner(klens)
    dev_args = runner.device_args(in_maps)
    LAST_EXEC = (runner, dev_args)
    results = runner.run(dev_args)
    LAST_RUN = results
    y = results[0]["y"].astype(np.float32)
    for c in range(1, NCORES):
        y = y + results[c]["y"].astype(np.float32)
    return y.reshape(1, S, HID)
